# revision 1
# baseline (speedup 1.0000x reference)
"""Trainium2 Bass kernel for nn_BTGRule (BTG rule scoring over a span chart).

Reference computation:
    L = span_rep @ Wl + bl            # [65, 65, 512]
    R = span_rep @ Wr + br            # [65, 65, 512]
    H = tanh(L[i, j] + R[j, k])       # over valid triples i < j < k
    scores[i, j, k] = H @ Wout + bout # [65, 65, 65, 2], zeros at invalid triples

Strategy (8 NeuronCores, SPMD — one program, per-core data):
  * Only valid (i, j) pairs (i < j <= 63) are computed; k runs over (j, 64].
  * Pairs are grouped by j into 8 groups g = ceil(j/8); group g has 8 j-slots
    (padded) and n = g pairs per core per j-slot, so every core runs an
    identical instruction stream. The i assignment per core is pure input data.
  * Host pre-transposes span_rep columns so every matmul has its contraction
    dim on partitions; the device never transposes. Projection inputs are
    fp16 (precision-equivalent to the f32r matmul's internal tf32-style
    rounding) and packed so each consumer needs exactly one DMA, ordered so
    compute starts ~3 us in instead of after the full ~11 us input stream.
  * On device per core: R^T projection (PE -> PSUM), fused broadcast add
    L + R via one DVE tensor_tensor per (group, h-tile) with step-0
    broadcast APs reading PSUM directly, per-section tanh right after each add (ACT —
    sections finish incrementally so score matmuls never wait a full-group
    tanh), score
    matmuls vs Wout in float32r (PE, full rate; fp32 proper is 4x slower),
    bias-add copies alternating DVE/ACT, one DMA out. A two-half software
    pipeline (all projections+tanh of a half, then its scores) keeps the
    in-order PE queue bubble-free.

Measured on 8 axon-attached TRN2 cores: ~55-61 us per invocation (in-NEFF
repeat-loop slope), global rel err ~4e-4 vs the fp32 reference.
"""

import numpy as np

N1 = 65          # chart side (N + 1)
HID = 512        # hidden size
OUT = 2          # output size
NCORES = 8
HT = HID // 128  # 4 h-tiles

# ---------------------------------------------------------------------------
# Pair-group layout (all compile-time constants, identical on host and device)
# ---------------------------------------------------------------------------
# group g (1..8): j in [8(g-1)+1, min(8g, 63)], padded to 8 j-slots.
# W[g] = max k-width in group = 63 - 8(g-1);  n[g] = g pairs per core per slot.


def _build_layout():
    groups = []
    r_off = 0   # column offset into the R / span_cols space (8*W per group)
    q_off = 0   # column offset into the Lsel / span_sel space (8*n per group)
    s_off = 0   # column offset into the S / output space (n*8*W per group)
    for g in range(1, 9):
        js = [8 * (g - 1) + 1 + t for t in range(8)]
        js = [j if j <= 63 else None for j in js]
        W = 63 - 8 * (g - 1)
        n = g
        groups.append(dict(g=g, js=js, W=W, n=n, r_off=r_off, q_off=q_off,
                           s_off=s_off))
        r_off += 8 * W
        q_off += 8 * n
        s_off += n * 8 * W
    return groups, r_off, q_off, s_off


GROUPS, RCOLS, QCOLS, SCOLS = _build_layout()  # 2240, 288, 7392

# R-projection chunks: merged so every float32r matmul has >=256 output cols
# (below 256 it runs 4 cycles/row). The last chunk is zero-padded to 256.
RCHUNK_IDXS = [[0], [1], [2], [3], [4, 5], [6, 7]]


def _build_rchunks():
    chunks = []
    off4 = 0
    for idxs in RCHUNK_IDXS:
        grps = [GROUPS[gi] for gi in idxs]
        rbase = grps[0]["r_off"]
        rcols = max(sum(8 * g["W"] for g in grps), 256)
        assert rcols <= 512
        chunks.append(dict(idxs=idxs, rbase=rbase, rcols=rcols, off4=off4))
        off4 += HT * rcols
    return chunks, off4


RCHUNKS, SPANP_COLS = _build_rchunks()   # packed span cols = 4 * sum(rcols)
# processing order: the small {g7,g8} chunk (256 cols) first so the first
# fused-add is gated by the least DMA; halves stay work-balanced
CHUNK_ORDER = [5, 0, 1, 2, 3, 4]

_COMPILED = None


def _build_program(reps=1):
    """Trace + compile the single SPMD program. reps>1 wraps the body in an
    on-device repeat loop (benchmarking only)."""
    import contextlib

    import concourse.bacc as bacc
    import concourse.mybir as mybir
    import concourse.tile as tile

    f32 = mybir.dt.float32
    f16 = mybir.dt.float16
    nc = bacc.Bacc("TRN2", target_bir_lowering=False, debug=False,
                   num_devices=NCORES)

    spanp_d = nc.declare_dram_parameter("spanp", [128, SPANP_COLS], f16,
                                        isOutput=False)
    selp_d = nc.declare_dram_parameter("selp", [128, HT * QCOLS], f16,
                                       isOutput=False)
    wp_d = nc.declare_dram_parameter("wp", [128, 2 * HT * HID], f16,
                                     isOutput=False)
    misc_d = nc.declare_dram_parameter("misc", [128, 16], f32, isOutput=False)
    out_d = nc.declare_dram_parameter("out", [OUT, SCOLS], f32, isOutput=True)

    ident = mybir.ActivationFunctionType.Identity
    tanh = mybir.ActivationFunctionType.Tanh

    # float32r: same 4-byte fp32 data, but the PE runs the matmul at full
    # rate (fp32 proper costs 4 cycles/row on TRN2).
    def r32(ap):
        return ap.bitcast(mybir.dt.float32r)

    def even_chunks(total, cap=512):
        # near-equal pieces, multiples of 8 (fp32r matmul ISA restriction
        # disallows odd output widths), each within one PSUM bank
        k = -(-total // cap)
        base = -(-total // (k * 8)) * 8
        return [base] * (k - 1) + [total - base * (k - 1)]

    with tile.TileContext(nc) as tc:
        with (
            tc.tile_pool(name="const", bufs=1) as cpool,
            tc.tile_pool(name="ps_r", bufs=5, space="PSUM") as ps_r,
            tc.tile_pool(name="ps_sc", bufs=3, space="PSUM") as ps_sc,
            tc.For_i(0, reps, 1, hint_engines=(mybir.EngineType.PE,
                                               mybir.EngineType.DVE,
                                               mybir.EngineType.Activation,
                                               mybir.EngineType.SP))
            if reps > 1 else contextlib.nullcontext(),
        ):
            # ---- input DMAs + interleaved warm-up --------------------------
            # Weights are packed as [128, (to*HT+ti)*128] blocks and DMA'd
            # per h_out so the Lsel and first R-projection matmuls can start
            # as soon as their own slices land. DMA issue order is chosen to
            # minimize the time until the first DVE broadcast-add.
            misc_t = cpool.tile([128, 16], f32, tag="misc")
            nc.sync.dma_start(r32(misc_t[:]), r32(misc_d[:]))
            blbr_t = misc_t[:, 0:HT]
            bout_t = misc_t[0:OUT, HT:HT + 1]
            woutp_t = misc_t[:, HT + 1:HT + 1 + OUT * HT]
            sel_t = cpool.tile([128, HT * QCOLS], f16, tag="sel")
            nc.sync.dma_start(sel_t[:], selp_d[:])
            w_t = cpool.tile([128, 2 * HT * HID], f16, tag="w")
            span_c = [None] * len(RCHUNKS)

            def dma_w(to):  # one DMA brings both Wl and Wr blocks for h_out=to
                nc.sync.dma_start(
                    w_t[:, to * 2 * HID:(to + 1) * 2 * HID],
                    wp_d[:, to * 2 * HID:(to + 1) * 2 * HID])

            def dma_span(ci):
                ch = RCHUNKS[ci]
                st = cpool.tile([128, HT * ch["rcols"]], f16, tag=f"spanc{ci}")
                nc.sync.dma_start(
                    st[:],
                    spanp_d[:, ch["off4"]:ch["off4"] + HT * ch["rcols"]])
                span_c[ci] = st

            dma_w(0)
            dma_span(CHUNK_ORDER[0])
            for to in range(1, HT):
                dma_w(to)
            for ci in CHUNK_ORDER[1:]:
                dma_span(ci)
            out_sb = cpool.tile([OUT, SCOLS], f32, tag="out")

            def wblk(kind, to, ti):  # kind 0 = Wl, 1 = Wr
                c0 = to * 2 * HID + kind * HID + ti * 128
                return w_t[:, c0:c0 + 128]

            # ---- Lsel(to) interleaved with R-chunk-0(to) -------------------
            lsel_t = []
            ch0 = RCHUNKS[CHUNK_ORDER[0]]
            pr0_tiles = []
            for to in range(HT):
                pl = ps_r.tile([128, QCOLS], f32, tag="psr")
                for ti in range(HT):
                    nc.tensor.matmul(
                        pl[:], wblk(0, to, ti),
                        sel_t[:, ti * QCOLS:(ti + 1) * QCOLS],
                        start=(ti == 0), stop=(ti == HT - 1))
                lt = cpool.tile([128, QCOLS], f32, tag=f"lsel{to}")
                nc.scalar.activation(lt[:], pl[:], ident,
                                     bias=blbr_t[:, to:to + 1])
                lsel_t.append(lt)
                pr = ps_r.tile([128, ch0["rcols"]], f32, tag="psr")
                for ti in range(HT):
                    nc.tensor.matmul(
                        pr[:], wblk(1, to, ti),
                        span_c[CHUNK_ORDER[0]][:, ti * ch0["rcols"]:
                                               (ti + 1) * ch0["rcols"]],
                        start=(ti == 0), stop=(ti == HT - 1))
                pr0_tiles.append(pr)

            # ---- two-half software pipeline --------------------------------
            # Per half: phase A = R projection (PE) -> fused broadcast add
            # (DVE, reads PSUM) -> tanh (ACT); phase B = score matmuls (PE)
            # -> bias-add copy (DVE). By the time the PE reaches a score
            # matmul its tanh finished while the PE ran other projections.
            ordered = [RCHUNKS[i] for i in CHUNK_ORDER]
            for half in (ordered[:4], ordered[4:]):
                s_tiles = {}
                pos = 0
                for ci, ch in enumerate(half):
                    rcols = ch["rcols"]
                    sc_t = span_c[RCHUNKS.index(ch)]
                    if ch is ch0:
                        pr_tiles = pr0_tiles
                    else:
                        pr_tiles = []
                        for to in range(HT):
                            pr = ps_r.tile([128, rcols], f32, tag="psr")
                            for ti in range(HT):
                                nc.tensor.matmul(
                                    pr[:], wblk(1, to, ti),
                                    sc_t[:, ti * rcols:(ti + 1) * rcols],
                                    start=(ti == 0), stop=(ti == HT - 1))
                            pr_tiles.append(pr)

                    for gi in ch["idxs"]:
                        grp = GROUPS[gi]
                        W, n = grp["W"], grp["n"]
                        q0 = grp["q_off"]
                        loc0 = grp["r_off"] - ch["rbase"]
                        cols = n * 8 * W
                        s = cpool.tile([128, HT * cols], f32, tag=f"s{pos}")
                        pos += 1
                        s_tiles[grp["g"]] = s
                        for to in range(HT):
                            out_v = (s[:, to * cols:(to + 1) * cols]
                                     .rearrange("p (a jj w) -> p a jj w",
                                                a=n, jj=8))
                            in0 = (pr_tiles[to][:, loc0:loc0 + 8 * W]
                                   .rearrange("p (jj w) -> p jj w", jj=8)
                                   .unsqueeze(1).broadcast_to([128, n, 8, W]))
                            in1 = (lsel_t[to][:, q0:q0 + 8 * n]
                                   .rearrange("p (a jj) -> p a jj", a=n)
                                   .unsqueeze(3).broadcast_to([128, n, 8, W]))
                            nc.vector.tensor_tensor(
                                out_v.bitcast(mybir.dt.float32r), in0, in1,
                                op=mybir.AluOpType.add)
                            # per-section tanh: score matmul `to` only needs
                            # section `to`, so finishing sections incrementally
                            # removes the 4.3us tanh-drain at phase boundaries
                            sec = s[:, to * cols:(to + 1) * cols]
                            nc.scalar.activation(r32(sec), r32(sec), tanh)

                # phase B: score matmuls + bias-add copies for this half
                for ch in half:
                    for gi in ch["idxs"]:
                        grp = GROUPS[gi]
                        W, n = grp["W"], grp["n"]
                        s0 = grp["s_off"]
                        cols = n * 8 * W
                        s = s_tiles[grp["g"]]
                        c = 0
                        for ci2, ccw in enumerate(even_chunks(cols)):
                            psc = ps_sc.tile([OUT, ccw], f32, tag="pssc")
                            for to in range(HT):
                                nc.tensor.matmul(
                                    psc[:],
                                    r32(woutp_t[:, OUT * to:OUT * (to + 1)]),
                                    r32(s[:, to * cols + c:
                                          to * cols + c + ccw]),
                                    start=(to == 0), stop=(to == HT - 1))
                            if (grp["g"] + ci2) % 2 == 0:
                                nc.vector.tensor_scalar_add(
                                    out_sb[:, s0 + c:s0 + c + ccw], psc[:],
                                    bout_t)
                            else:
                                nc.scalar.activation(
                                    out_sb[:, s0 + c:s0 + c + ccw], psc[:],
                                    ident, bias=bout_t)
                            c += ccw

            nc.sync.dma_start(out_d[:], out_sb[:])

    nc.compile()
    return nc


def _get_compiled():
    global _COMPILED
    if _COMPILED is None:
        _COMPILED = _build_program()
    return _COMPILED


# ---------------------------------------------------------------------------
# Host-side sharding / unsharding
# ---------------------------------------------------------------------------

def make_inputs(span_rep, Wl, bl, Wr, br, Wout, bout):
    """Build the per-core input maps (packed layouts, see _build_program)."""
    span_rep = np.ascontiguousarray(np.asarray(span_rep, dtype=np.float32))
    Wl = np.ascontiguousarray(np.asarray(Wl, dtype=np.float32))
    Wr = np.ascontiguousarray(np.asarray(Wr, dtype=np.float32))
    Wout = np.asarray(Wout, dtype=np.float32)
    bl = np.asarray(bl, dtype=np.float32)
    br = np.asarray(br, dtype=np.float32)
    bout = np.asarray(bout, dtype=np.float32)

    # span columns in the flat (r_off) space: span_rep[j, k]^T per j-slot
    span_cols = np.zeros((HID, RCOLS + 80), dtype=np.float32)  # +tail pad
    for grp in GROUPS:
        W = grp["W"]
        for jj, j in enumerate(grp["js"]):
            if j is None:
                continue
            w = 64 - j
            c0 = grp["r_off"] + jj * W
            span_cols[:, c0:c0 + w] = span_rep[j, j + 1:65, :].T

    # packed per-chunk, h-tile-major span: [128, HT * rcols per chunk]
    spanp = np.zeros((128, SPANP_COLS), dtype=np.float16)
    for ch in RCHUNKS:
        for ti in range(HT):
            blk = span_cols[ti * 128:(ti + 1) * 128,
                            ch["rbase"]:ch["rbase"] + ch["rcols"]]
            spanp[:, ch["off4"] + ti * ch["rcols"]:
                  ch["off4"] + (ti + 1) * ch["rcols"]] = blk

    def pack_ht(M, width):  # [512, width] -> [128, HT*width], h-tile-major
        out = np.empty((128, HT * width), dtype=np.float16)
        for ti in range(HT):
            out[:, ti * width:(ti + 1) * width] = M[ti * 128:(ti + 1) * 128, :]
        return out

    # weights packed as [128, 2*HT*HID]: per h_out block `to`, Wl's four
    # h_in 128-blocks then Wr's four
    wp = np.empty((128, 2 * HT * HID), dtype=np.float16)
    for to in range(HT):
        for kind, M in ((0, Wl), (1, Wr)):
            for ti in range(HT):
                c0 = to * 2 * HID + kind * HID + ti * 128
                wp[:, c0:c0 + 128] = \
                    M[ti * 128:(ti + 1) * 128, to * 128:(to + 1) * 128]

    # span_sel: per core. Column q = a*8 + jj of group g holds
    # span_rep[i, j]^T with i = a*8 + core (if that is a valid pair).
    selps = []
    for core in range(NCORES):
        sel = np.zeros((HID, QCOLS), dtype=np.float32)
        for grp in GROUPS:
            n = grp["n"]
            for jj, j in enumerate(grp["js"]):
                if j is None:
                    continue
                for a in range(n):
                    i = a * 8 + core
                    if i < j:
                        sel[:, grp["q_off"] + a * 8 + jj] = span_rep[i, j, :]
        selps.append(pack_ht(sel, QCOLS))

    misc = np.zeros((128, 16), dtype=np.float32)
    misc[:, 0:HT] = (bl + br).reshape(HT, 128).T
    misc[0:OUT, HT] = bout
    misc[:, HT + 1:HT + 1 + OUT * HT] = (
        Wout.reshape(HT, 128, OUT).transpose(1, 0, 2).reshape(128, HT * OUT))

    in_maps = []
    for core in range(NCORES):
        in_maps.append({
            "spanp": spanp,
            "selp": selps[core],
            "wp": wp,
            "misc": misc,
        })
    return in_maps


def scatter_outputs(core_outs):
    """Assemble the full [65, 65, 65, 2] output from per-core [2, SCOLS]."""
    full = np.zeros((N1, N1, N1, OUT), dtype=np.float32)
    for core in range(NCORES):
        oc = core_outs[core]
        for grp in GROUPS:
            W, n = grp["W"], grp["n"]
            for jj, j in enumerate(grp["js"]):
                if j is None:
                    continue
                w = 64 - j
                for a in range(n):
                    i = a * 8 + core
                    if i < j:
                        c0 = grp["s_off"] + (a * 8 + jj) * W
                        full[i, j, j + 1:65, :] = oc[:, c0:c0 + w].T
    return full


def kernel(span_rep, Wl, bl, Wr, br, Wout, bout):
    from concourse.bass_utils import run_bass_kernel_spmd

    nc = _get_compiled()
    in_maps = make_inputs(span_rep, Wl, bl, Wr, br, Wout, bout)
    res = run_bass_kernel_spmd(nc, in_maps, core_ids=list(range(NCORES)))
    core_outs = [res.results[c]["out"] for c in range(NCORES)]
    return scatter_outputs(core_outs)


if __name__ == "__main__":
    rng = np.random.default_rng(0)
    s = 1.0 / np.sqrt(HID)
    inputs = dict(
        span_rep=rng.standard_normal((N1, N1, HID)).astype(np.float32),
        Wl=(rng.standard_normal((HID, HID)) * s).astype(np.float32),
        bl=np.zeros(HID, np.float32),
        Wr=(rng.standard_normal((HID, HID)) * s).astype(np.float32),
        br=np.zeros(HID, np.float32),
        Wout=(rng.standard_normal((HID, OUT)) * s).astype(np.float32),
        bout=np.zeros(OUT, np.float32),
    )
    out = kernel(**inputs)
    print("out", out.shape, out.dtype, np.abs(out).max())



# revision 2
# speedup vs baseline: 1.1353x; 1.1353x over previous
"""Trainium2 Bass kernel for nn_BTGRule — j-sharded slotted design (v2).

Reference computation:
    L = span_rep @ Wl + bl            # [65, 65, 512]
    R = span_rep @ Wr + br            # [65, 65, 512]
    H = tanh(L[i, j] + R[j, k])       # over valid triples i < j < k
    scores[i, j, k] = H @ Wout + bout # [65, 65, 65, 2]

Sharding: split-point axis j is sharded across the 8 cores (each j's whole
(i, k) block lives on one core), so the L/R projections are computed once
total instead of once per core.  SPMD needs identical instruction streams,
so work is organized in 8 compile-time SLOTS of shape (A_s, W_s) =
(4(s+1), 64-4s).  A j-block of shape (j, 64-j) fits slot s with a = j
(normal orientation) or, transposed, a = 64-j.  Cores 0-3 take the normal
j in [1,32], cores 4-7 the transposed j in [33,63]; orientation is uniform
per core, so the only per-core difference is pure data: which spans go in
the "dense" (broadcast over a) vs "column" (broadcast over w) operand and
the order (Wr|Wl) vs (Wl|Wr) of the packed weights.

Per core per rep:
  PE:  dense/column projections per (slot-pair, hout) into one PSUM bank
       (+ bias via a 1-row ones matmul), then score matmuls vs Wout.
  DVE: one PSUM->SBUF f16 copy per (pair, hout); fused broadcast-add
       L+R per (slot, hout) via tensor_tensor with packed-pair APs
       (both operands 2-byte, last AP dim [1,2] -> 2x mode).
  ACT: tanh per pair (large FD), bias-free; half the score copies.
  Host: packs spans/weights, scatters [2, 6528] per core to the dense
       [65,65,65,2] output and adds bout there.
"""

import numpy as np

N1 = 65
HID = 512
HT = 4            # 128-row h tiles
OUT = 2
NCORES = 8

# slots: s -> (A, W); pair p couples slots (p, 7-p); physical order in H
SLOTS = [(4 * (s + 1), 64 - 4 * s) for s in range(8)]
PAIRS = [(0, 7), (1, 6), (2, 5), (3, 4)]
ORDER = [1, 6, 7, 5, 4, 3, 2, 0]        # slot processing order
SEQ = ORDER                              # H/out layout = processing order
# span DMA layout: pairs in first-use order
PAIR_USE = []
for _s in ORDER:
    _p = _s if _s <= 3 else 7 - _s
    if _p not in PAIR_USE:
        PAIR_USE.append(_p)
PAIR_POS = {p: i for i, p in enumerate(PAIR_USE)}
DW = 100          # dense cols per pair (W_sa + W_sb), same for all pairs
DC = 72           # doubled column cols per pair (2*(A_sa+A_sb))
SC = sum(a * w for a, w in SLOTS)        # 6528 H cols per core

# offsets in SEQ layout
_d_off, _c_off, _h_off, _s_off = {}, {}, {}, {}
_d, _c, _h = 0, 0, 0
for s in SEQ:
    A, W = SLOTS[s]
    _d_off[s], _c_off[s], _h_off[s], _s_off[s] = _d, _c, _h, _h
    _d += W
    _c += 2 * A
    _h += A * W
assert _d == 4 * DW and _c == 4 * DC and _h == SC

# pair offsets (contiguous in SEQ layout)
PAIR_D = {p: _d_off[sa] for p, (sa, sb) in enumerate(PAIRS)}
PAIR_C = {p: _c_off[sa] for p, (sa, sb) in enumerate(PAIRS)}
PAIR_H = {p: _h_off[sa] for p, (sa, sb) in enumerate(PAIRS)}


def _chunks(cols):
    n = -(-cols // 512)
    base = -(-cols // (2 * n)) * 2
    out = [base] * (n - 1) + [cols - base * (n - 1)]
    assert all(c % 2 == 0 and 0 < c <= 512 for c in out)
    return out


def jmap(core, s):
    """j hosted by (core, slot); None for the one dummy instance."""
    if core < 4:
        return 4 * s + 1 + core                 # normal, j in [1, 32]
    a = 4 * s + 1 + (core - 4)                  # transposed, a = 64 - j
    j = 64 - a
    return j if j >= 33 else None               # (core 7, slot 7) dummy


_COMPILED = None


def _build_program(reps=1):
    import contextlib

    import concourse.bacc as bacc
    import concourse.mybir as mybir
    import concourse.tile as tile

    f32 = mybir.dt.float32
    f16 = mybir.dt.float16
    tanh = mybir.ActivationFunctionType.Tanh
    ident = mybir.ActivationFunctionType.Identity
    add = mybir.AluOpType.add

    nc = bacc.Bacc("TRN2", target_bir_lowering=False, debug=False,
                   num_devices=NCORES)

    wp_d = nc.declare_dram_parameter("wp", [128, 2 * HT * HID], f16,
                                     isOutput=False)
    spd_d = nc.declare_dram_parameter("spd", [128, HT * 4 * DW], f16,
                                      isOutput=False)
    spc_d = nc.declare_dram_parameter("spc", [128, HT * 4 * DC], f16,
                                      isOutput=False)
    misc_d = nc.declare_dram_parameter("misc", [128, HID + HT * OUT], f16,
                                       isOutput=False)
    out_d = nc.declare_dram_parameter("out", [OUT, SC], f32, isOutput=True)

    with tile.TileContext(nc) as tc:
        with (
            tc.tile_pool(name="const", bufs=1) as cpool,
            tc.tile_pool(name="stream", bufs=2) as spool,
            tc.tile_pool(name="ps_pr", bufs=3, space="PSUM") as ps_pr,
            tc.tile_pool(name="ps_sc", bufs=4, space="PSUM") as ps_sc,
            tc.For_i(0, reps, 1, hint_engines=(mybir.EngineType.PE,
                                               mybir.EngineType.DVE,
                                               mybir.EngineType.Activation,
                                               mybir.EngineType.SP))
            if reps > 1 else contextlib.nullcontext(),
        ):
            misc_t = spool.tile([128, HID + HT * OUT], f16, tag="misc")
            blbr_t = misc_t[0:1, 0:HID]
            wout_t = misc_t[:, HID:HID + HT * OUT]
            wp_t = spool.tile([128, 2 * HT * HID], f16, tag="wp")
            spd_t = spool.tile([128, HT * 4 * DW], f16, tag="spd")
            spc_t = spool.tile([128, HT * 4 * DC], f16, tag="spc")
            # spans: two halves (use-order positions 0-1 then 2-3), sync ring
            nc.sync.dma_start(spd_t[:, 0:2 * HT * DW], spd_d[:, 0:2 * HT * DW])
            nc.sync.dma_start(spc_t[:, 0:2 * HT * DC], spc_d[:, 0:2 * HT * DC])
            nc.sync.dma_start(spd_t[:, 2 * HT * DW:4 * HT * DW],
                              spd_d[:, 2 * HT * DW:4 * HT * DW])
            nc.sync.dma_start(spc_t[:, 2 * HT * DC:4 * HT * DC],
                              spc_d[:, 2 * HT * DC:4 * HT * DC])
            # weights + misc on the scalar HWDGE ring (parallel issue)
            for t in range(HT):
                nc.scalar.dma_start(wp_t[:, t * 1024:(t + 1) * 1024],
                                    wp_d[:, t * 1024:(t + 1) * 1024])
            nc.scalar.dma_start(misc_t[:], misc_d[:])
            ones_t = cpool.tile([1, DC], f16, tag="ones")
            nc.vector.memset(ones_t[:], 1.0)

            # weight block: kind 0 = dense, 1 = column; hout t; hin hi
            def wblk(kind, t, hi):
                c0 = t * 1024 + kind * HID + hi * 128
                return wp_t[:, c0:c0 + 128]

            sbDC = spool.tile([128, 4 * HT * (DW + DC)], f16, tag="sbDC")
            H_t = cpool.tile([128, HT * SC], f16, tag="H")
            out_sb = spool.tile([OUT, SC], f32, tag="osb")

            def proj_pair(p, houts=range(HT)):
                # projections for both slots of pair p
                for t in houts:
                    ps = ps_pr.tile([128, DW + DC], f32, tag="pspr")
                    for hi in range(HT):
                        nc.tensor.matmul(
                            ps[:, 0:DW], wblk(0, t, hi),
                            spd_t[:, (PAIR_POS[p] * HT + hi) * DW:
                                  (PAIR_POS[p] * HT + hi) * DW + DW],
                            start=(hi == 0), stop=(hi == HT - 1))
                    for hi in range(HT):
                        nc.tensor.matmul(
                            ps[:, DW:DW + DC], wblk(1, t, hi),
                            spc_t[:, (PAIR_POS[p] * HT + hi) * DC:
                                  (PAIR_POS[p] * HT + hi) * DC + DC],
                            start=(hi == 0), stop=False)
                    nc.tensor.matmul(
                        ps[:, DW:DW + DC], blbr_t[0:1, t * 128:(t + 1) * 128],
                        ones_t[0:1, :], start=False, stop=True)
                    # one merged PSUM->SBUF f16 copy per (pair, hout)
                    g0 = (p * HT + t) * (DW + DC)
                    nc.vector.tensor_copy(sbDC[:, g0:g0 + DW + DC], ps[:])

            def adds_pair(p, only_slot=None):
                for si, s in enumerate(PAIRS[p]):
                    if only_slot is not None and s != only_slot:
                        continue
                    A, W = SLOTS[s]
                    dd = 0 if si == 0 else SLOTS[PAIRS[p][0]][1]
                    cc = DW if si == 0 else DW + 2 * SLOTS[PAIRS[p][0]][0]
                    for t in range(HT):
                        h0 = HT * _h_off[s] + t * A * W
                        out_v = (H_t[:, h0:h0 + A * W]
                                 .rearrange("p (a w2 two) -> p a w2 two",
                                            a=A, two=2))
                        g0 = (p * HT + t) * (DW + DC)
                        in0 = (sbDC[:, g0 + dd:g0 + dd + W]
                               .rearrange("p (w2 two) -> p w2 two", two=2)
                               .unsqueeze(1)
                               .broadcast_to([128, A, W // 2, 2]))
                        in1 = (sbDC[:, g0 + cc:g0 + cc + 2 * A]
                               .rearrange("p (a two) -> p a two", two=2)
                               .unsqueeze(2)
                               .broadcast_to([128, A, W // 2, 2]))
                        nc.vector.tensor_tensor(out_v, in0, in1, op=add)

            def tanh_slot(s):
                h0 = HT * _h_off[s]
                n = HT * SLOTS[s][0] * SLOTS[s][1]
                sec = H_t[:, h0:h0 + n]
                nc.scalar.activation(sec, sec, tanh)

            def tanh_pair(p):
                sa, sb = PAIRS[p]
                h0 = HT * PAIR_H[p]
                n = HT * (SLOTS[sa][0] * SLOTS[sa][1]
                          + SLOTS[sb][0] * SLOTS[sb][1])
                sec = H_t[:, h0:h0 + n]
                nc.scalar.activation(sec, sec, tanh)

            def scores_slot(s, outcnt=[0]):
                    A, W = SLOTS[s]
                    cols = A * W
                    c = 0
                    for ccw in _chunks(cols):
                        psc = ps_sc.tile([OUT, ccw], f32, tag="pssc")
                        for t in range(HT):
                            h0 = HT * _h_off[s] + t * cols
                            nc.tensor.matmul(
                                psc[:], wout_t[:, OUT * t:OUT * (t + 1)],
                                H_t[:, h0 + c:h0 + c + ccw],
                                start=(t == 0), stop=(t == HT - 1))
                        dst = out_sb[:, _s_off[s] + c:_s_off[s] + c + ccw]
                        if outcnt[0] % 2 == 0:
                            nc.vector.tensor_copy(dst, psc[:])
                        else:
                            nc.scalar.activation(dst, psc[:], ident)
                        outcnt[0] += 1
                        c += ccw

            def scores_pair(p, outcnt=[0]):
                for s in PAIRS[p]:
                    A, W = SLOTS[s]
                    cols = A * W
                    c = 0
                    for ccw in _chunks(cols):
                        psc = ps_sc.tile([OUT, ccw], f32, tag="pssc")
                        for t in range(HT):
                            h0 = HT * _h_off[s] + t * cols
                            nc.tensor.matmul(
                                psc[:], wout_t[:, OUT * t:OUT * (t + 1)],
                                H_t[:, h0 + c:h0 + c + ccw],
                                start=(t == 0), stop=(t == HT - 1))
                        dst = out_sb[:, _s_off[s] + c:_s_off[s] + c + ccw]
                        if outcnt[0] % 2 == 0:
                            nc.vector.tensor_copy(dst, psc[:])
                        else:
                            nc.scalar.activation(dst, psc[:], ident)
                        outcnt[0] += 1
                        c += ccw

            # slot-granular pipeline over ORDER
            pair_of = {s2: p for p, pr in enumerate(PAIRS) for s2 in pr}
            seen = set()

            def need(s2):
                p = pair_of[s2]
                if p not in seen:
                    seen.add(p)
                    proj_pair(p)

            o = ORDER
            need(o[0])
            adds_pair(pair_of[o[0]], only_slot=o[0])
            need(o[1])
            tanh_slot(o[0])
            adds_pair(pair_of[o[1]], only_slot=o[1])
            for k in range(2, 8):
                need(o[k])
                tanh_slot(o[k - 1])
                scores_slot(o[k - 2])
                adds_pair(pair_of[o[k]], only_slot=o[k])
            tanh_slot(o[7])
            scores_slot(o[6])
            cut = _s_off[o[6]]      # slots o[0..5] are laid out before o[6]
            nc.sync.dma_start(out_d[:, 0:cut], out_sb[:, 0:cut])
            scores_slot(o[7])
            nc.sync.dma_start(out_d[:, cut:], out_sb[:, cut:])

    nc.compile()
    return nc


def _get_compiled():
    global _COMPILED
    if _COMPILED is None:
        _COMPILED = _build_program()
    return _COMPILED


# ---------------------------------------------------------------------------
# Host-side packing / scatter
# ---------------------------------------------------------------------------

def make_inputs(span_rep, Wl, bl, Wr, br, Wout, bout):
    span_rep = np.ascontiguousarray(np.asarray(span_rep, np.float32))
    Wl = np.asarray(Wl, np.float32)
    Wr = np.asarray(Wr, np.float32)
    Wout = np.asarray(Wout, np.float32)
    blbr = (np.asarray(bl, np.float32) + np.asarray(br, np.float32))

    def pack_ht(M, width):      # [512, width] f32 -> [128, HT*width] f16
        o = np.empty((128, HT * width), np.float16)
        for hi in range(HT):
            o[:, hi * width:(hi + 1) * width] = M[hi * 128:(hi + 1) * 128]
        return o

    def pack_w(Wd, Wc):         # [128, 2*HT*HID] f16
        o = np.empty((128, 2 * HT * HID), np.float16)
        for t in range(HT):
            for kind, M in ((0, Wd), (1, Wc)):
                for hi in range(HT):
                    c0 = t * 1024 + kind * HID + hi * 128
                    o[:, c0:c0 + 128] = \
                        M[hi * 128:(hi + 1) * 128, t * 128:(t + 1) * 128]
        return o

    wp_n = pack_w(Wr, Wl)       # normal cores: dense=R(Wr), col=L(Wl)
    wp_t = pack_w(Wl, Wr)       # transposed:   dense=L(Wl), col=R(Wr)
    misc = np.zeros((128, HID + HT * OUT), np.float16)
    misc[0, 0:HID] = blbr.astype(np.float16)
    for t in range(HT):
        misc[:, HID + OUT * t:HID + OUT * (t + 1)] = Wout[t * 128:(t + 1) * 128]

    in_maps = []
    for core in range(NCORES):
        # pair-major packing: [pair][hin-block][cols]
        spd = np.zeros((128, HT * 4 * DW), np.float16)
        spc = np.zeros((128, HT * 4 * DC), np.float16)
        for p, (sa, sb) in enumerate(PAIRS):
            ppos = PAIR_POS[p]
            dblk = np.zeros((HID, DW), np.float32)
            cblk = np.zeros((HID, DC), np.float32)
            for si, s in enumerate((sa, sb)):
                j = jmap(core, s)
                if j is None:
                    continue
                if core < 4:    # normal: a=i (count j), w=k (count 64-j)
                    dn = span_rep[j, j + 1:65].T       # [512, 64-j]
                    cn = span_rep[0:j, j].T            # [512, j]
                else:           # transposed: a=k, w=i
                    dn = span_rep[0:j, j].T            # [512, j]
                    cn = span_rep[j, j + 1:65].T       # [512, 64-j]
                dd = 0 if si == 0 else SLOTS[sa][1]
                cc = 0 if si == 0 else 2 * SLOTS[sa][0]
                dblk[:, dd:dd + dn.shape[1]] = dn
                cblk[:, cc:cc + 2 * cn.shape[1]:2] = cn
                cblk[:, cc + 1:cc + 2 * cn.shape[1]:2] = cn
            for hi in range(HT):
                spd[:, (ppos * HT + hi) * DW:(ppos * HT + hi + 1) * DW] = \
                    dblk[hi * 128:(hi + 1) * 128]
                spc[:, (ppos * HT + hi) * DC:(ppos * HT + hi + 1) * DC] = \
                    cblk[hi * 128:(hi + 1) * 128]
        in_maps.append({
            "wp": wp_n if core < 4 else wp_t,
            "spd": spd,
            "spc": spc,
            "misc": misc,
        })
    return in_maps


def scatter_outputs(core_outs, bout):
    bout = np.asarray(bout, np.float32)
    full = np.zeros((N1, N1, N1, OUT), np.float32)
    for core in range(NCORES):
        oc = np.asarray(core_outs[core])
        for s in range(8):
            j = jmap(core, s)
            if j is None:
                continue
            A, W = SLOTS[s]
            blk = oc[:, _s_off[s]:_s_off[s] + A * W].reshape(OUT, A, W)
            if core < 4:
                full[0:j, j, j + 1:65, :] = \
                    blk[:, 0:j, 0:64 - j].transpose(1, 2, 0) + bout
            else:
                full[0:j, j, j + 1:65, :] = \
                    blk[:, 0:64 - j, 0:j].transpose(2, 1, 0) + bout
    return full


def kernel(span_rep, Wl, bl, Wr, br, Wout, bout):
    from concourse.bass_utils import run_bass_kernel_spmd

    nc = _get_compiled()
    in_maps = make_inputs(span_rep, Wl, bl, Wr, br, Wout, bout)
    res = run_bass_kernel_spmd(nc, in_maps, core_ids=list(range(NCORES)))
    core_outs = [res.results[c]["out"] for c in range(NCORES)]
    return scatter_outputs(core_outs, bout)


if __name__ == "__main__":
    rng = np.random.default_rng(0)
    s = 1.0 / np.sqrt(HID)
    inputs = dict(
        span_rep=rng.standard_normal((N1, N1, HID)).astype(np.float32),
        Wl=(rng.standard_normal((HID, HID)) * s).astype(np.float32),
        bl=np.zeros(HID, np.float32),
        Wr=(rng.standard_normal((HID, HID)) * s).astype(np.float32),
        br=np.zeros(HID, np.float32),
        Wout=(rng.standard_normal((HID, OUT)) * s).astype(np.float32),
        bout=np.zeros(OUT, np.float32),
    )
    out = kernel(**inputs)
    print("out", out.shape, out.dtype, np.abs(out).max())

    # host-side check against a numpy reference
    L = inputs["span_rep"] @ inputs["Wl"] + inputs["bl"]
    R = inputs["span_rep"] @ inputs["Wr"] + inputs["br"]
    idx = np.arange(N1)
    valid = (idx[:, None, None] < idx[None, :, None]) & \
            (idx[None, :, None] < idx[None, None, :])
    Hf = np.tanh(L[:, :, None, :] + R[None, :, :, :])
    exp = (Hf @ inputs["Wout"] + inputs["bout"]) * valid[..., None]
    rel = np.abs(out - exp).max() / np.abs(exp).max()
    print("rel err vs numpy reference:", rel)
